# revision 22
# baseline (speedup 1.0000x reference)
"""Causal self-attention (B=4, T=2048, C=1024, H=16) on 8 Trainium2 cores.

Sharding: core c -> batch b = c//2, head-group g = c%2 (8 heads each,
tensor-parallel). QKV + attention + c_proj computed per core on its head
slice; partial c_proj outputs of a (b) pair are summed with chunked
on-device ReduceScatters over the T dimension; host reassembles.

x arrives pre-transposed/pre-cast ([C, T] bf16) from the host, so the
kernel starts matmuls as soon as the first t-slice lands. Attention is
pipelined per (head-pair, 1024-query-block) unit: score matmuls (PE) ->
exp (Scalar, both heads per ACT) -> causal diag zeroing (GpSimd) ->
p@v chains (PE) -> normalize (Vector recip + GpSimd mul) -> y^T
(PE transpose + GpSimd copy). Projection + ReduceScatter fire in 8
chunks of 256 rows as soon as each row range is complete.

Self-contained: only imports concourse (installed library) + numpy.
"""

import ml_dtypes
import numpy as np

import concourse.mybir as mybir
import concourse.tile as tile
from concourse import bacc
from concourse.bass_utils import run_bass_kernel_spmd
from concourse.masks import make_identity

B, T, C = 4, 2048, 1024
H_TOTAL, D = 16, 64
N_CORES = 8
HL = H_TOTAL // 2  # local heads per core (8)
HC = HL * D  # local head cols (512)
NP = HL // 2  # head pairs (4)
P = 128
TT = T // P  # 16 t-chunks of 128
CK = C // P  # 8 contraction chunks for qkv
WIN = 512
NW = T // WIN  # 4 query windows of 512
NRC = 8  # ReduceScatter chunks (256 rows in, 128 out each)
F32 = mybir.dt.float32
BF16 = mybir.dt.bfloat16
SCALE = 1.0 / 8.0  # 1/sqrt(D)
MASK_VAL = -480.0  # -60 after the 1/8 attention scale; exp(-60) ~ 0

_CACHE = {}


def _build_nc():
    nc = bacc.Bacc("TRN2", target_bir_lowering=False, debug=False, num_devices=N_CORES)

    xT_d = nc.dram_tensor("xT", [NW, P, CK, WIN], BF16, kind="ExternalInput")
    wq_d = nc.dram_tensor("wq", [P, NP, CK, P], BF16, kind="ExternalInput")
    wk_d = nc.dram_tensor("wk", [P, NP, CK, P], BF16, kind="ExternalInput")
    wv_d = nc.dram_tensor("wv", [P, CK, HC], BF16, kind="ExternalInput")
    bq_d = nc.dram_tensor("bq", [P, NP], F32, kind="ExternalInput")
    bk_d = nc.dram_tensor("bk", [P, NP], F32, kind="ExternalInput")
    bv_d = nc.dram_tensor("bv", [P, HC], BF16, kind="ExternalInput")
    wp_d = nc.dram_tensor("wp", [P, HC // P, C], BF16, kind="ExternalInput")
    bp_d = nc.dram_tensor("bp", [P, C], BF16, kind="ExternalInput")
    out_d = nc.dram_tensor("out", [T // 2, C], BF16, kind="ExternalOutput")

    with tile.TileContext(nc) as tc:
        with (
            tc.tile_pool(name="const", bufs=1) as constp,
            tc.tile_pool(name="big", bufs=1) as bigp,
            tc.tile_pool(name="pp", bufs=2) as pp,
            tc.tile_pool(name="small", bufs=2) as smallp,
            tc.tile_pool(name="zout", bufs=2) as zoutp,
            tc.tile_pool(name="score_ps", bufs=2, space="PSUM") as score_ps,
            tc.tile_pool(name="av_ps", bufs=2, space="PSUM") as av_ps,
            tc.tile_pool(name="mm_ps", bufs=2, space="PSUM") as mm_ps,
            tc.tile_pool(name="dram", bufs=1, space="DRAM") as dramp,
        ):
            # ---- constants ----
            ident = constp.tile([P, P], F32)
            make_identity(nc, ident)
            ident_bf = constp.tile([P, P], BF16)
            nc.vector.tensor_copy(out=ident_bf[:], in_=ident[:])
            # multiplicative causal mask for the diagonal 128x128 block:
            # trimask[k, q] = 1 where q >= k else 0
            trif = constp.tile([P, P], F32)
            nc.gpsimd.memset(trif, 1.0)
            nc.gpsimd.affine_select(
                out=trif,
                in_=trif,
                compare_op=mybir.AluOpType.is_ge,
                fill=0.0,
                base=0,
                pattern=[[1, P]],
                channel_multiplier=-1,
            )
            trimask = constp.tile([P, P], BF16)
            nc.vector.tensor_copy(out=trimask[:], in_=trif[:])
            bq_sb = constp.tile([P, NP], F32)
            bk_sb = constp.tile([P, NP], F32)
            bv_sb = constp.tile([P, HC], BF16)
            bp_sb = constp.tile([P, C], BF16)

            # ---- persistent activations / weights ----
            xT = bigp.tile([P, NW, CK, WIN], BF16)  # x^T [c, (slice, ck, t)]
            qT = bigp.tile([P, NP, T], BF16)  # q^T [qcol, t]
            kT = bigp.tile([P, NP, T], BF16)  # k^T [kcol, t]
            v_ext = bigp.tile([P, TT, HL, D + 1], BF16)  # v with ones col
            yT = bigp.tile([P, NP, T], BF16)  # y^T [ci, t]
            wv_sb = bigp.tile([P, CK, HC], BF16)
            wp_sb = bigp.tile([P, HC // P, C], BF16)
            wq_sb = bigp.tile([P, NP, CK, P], BF16)
            wk_sb = bigp.tile([P, NP, CK, P], BF16)

            # spread the startup DMAs over independent engine DGE rings so
            # the first v/qk chains are gated only by xT slice 0 + its weights:
            # scalar ring: wv first; gpsimd ring: wq first; sync ring: xT slices
            nc.scalar.dma_start(wv_sb[:], wv_d[:])
            nc.gpsimd.dma_start(wq_sb[:], wq_d[:])
            nc.scalar.dma_start(wk_sb[:], wk_d[:])
            nc.gpsimd.dma_start(wp_sb[:], wp_d[:])
            nc.scalar.dma_start(bv_sb[:], bv_d[:])
            nc.scalar.dma_start(bq_sb[:], bq_d[:])
            nc.scalar.dma_start(bk_sb[:], bk_d[:])
            nc.gpsimd.dma_start(bp_sb[:], bp_d[:])
            nc.vector.memset(v_ext[:, :, :, D : D + 1], 1.0)

            # one DRAM staging tensor per RS chunk so each collective
            # depends only on its own rows' writes
            rs_rows = ((0, 1024), (1024, 512), (1536, 256), (1792, 256))
            z_drams = {
                r0: dramp.tile([nr, C], BF16, name=f"z_dram{r0}")
                for r0, nr in rs_rows
            }
            rs_out = dramp.tile([T // 2, C], BF16)

            def z_chunk(row0):
                for r0, nr in rs_rows:
                    if r0 <= row0 < r0 + nr:
                        return z_drams[r0], row0 - r0
                raise AssertionError

            # ---- phase 1: qkv projections, pipelined on xT t-slices ----
            for s in range(NW):
                nc.sync.dma_start(xT[:, s], xT_d[s])
                for tt in range(4 * s, 4 * s + 4):
                    ps = mm_ps.tile([P, 512], F32, tag="mm", name=f"v_ps{tt}")
                    for ck in range(CK):
                        nc.tensor.matmul(
                            ps[:],
                            xT[:, tt // 4, ck, (tt % 4) * P : (tt % 4 + 1) * P],
                            wv_sb[:, ck, :],
                            start=(ck == 0),
                            stop=(ck == CK - 1),
                        )
                    nc.vector.tensor_add(
                        out=v_ext[:, tt, :, 0:D],
                        in0=ps[:].rearrange("p (h d) -> p h d", d=D),
                        in1=bv_sb[:].rearrange("p (h d) -> p h d", d=D),
                    )
                for j in range(NP):
                    for w_sb, b_sb, dstT in (
                        (wq_sb, bq_sb, qT),
                        (wk_sb, bk_sb, kT),
                    ):
                        ps = mm_ps.tile([P, 512], F32, tag="mm", name=f"qk_ps{s}{j}")
                        for ck in range(CK):
                            nc.tensor.matmul(
                                ps[:],
                                w_sb[:, j, ck, :],
                                xT[:, s, ck, :],
                                start=(ck == 0),
                                stop=(ck == CK - 1),
                            )
                        nc.vector.tensor_add(
                            out=dstT[:, j, s * WIN : (s + 1) * WIN],
                            in0=ps[:],
                            in1=b_sb[:, j : j + 1].to_broadcast((P, WIN)),
                        )

            # ---- attention units + chunked proj/RS ----
            # unit = (query-window w of 512, head-pair j); keys <= (w+1)*512
            def unit(j, w):
                qw = w * WIN
                n_i = 4 * (w + 1)
                p = pp.tile(
                    [P, TT, 2, WIN], BF16, tag="p", name=f"p{j}{w}"
                )
                for i in range(n_i):
                    c0r = i * P - qw
                    c0 = max(0, c0r)
                    sp = score_ps.tile(
                        [P, 2, WIN], F32, tag="sc", name=f"sp{j}{w}{i}"
                    )
                    for h in range(2):
                        hb = h * D
                        nc.tensor.matmul(
                            sp[:, h, c0:WIN],
                            kT[hb : hb + D, j, i * P : (i + 1) * P],
                            qT[hb : hb + D, j, qw + c0 : qw + WIN],
                            start=True,
                            stop=True,
                        )
                    nc.scalar.activation(
                        out=p[:, i, :, c0:WIN],
                        in_=sp[:, :, c0:WIN],
                        func=mybir.ActivationFunctionType.Exp,
                        scale=SCALE,
                    )
                    if c0r >= 0:
                        # zero the strictly-upper triangle of the
                        # diagonal block (future keys) post-exp
                        nc.vector.tensor_mul(
                            out=p[:, i, :, c0 : c0 + P],
                            in0=p[:, i, :, c0 : c0 + P],
                            in1=trimask[:, None, :].to_broadcast((P, 2, P)),
                        )
                for tl in range(4):
                    tg = w * 4 + tl
                    av = av_ps.tile([P, 2, D + 1], F32, tag="av", name=f"av{j}{w}{tl}")
                    for h in range(2):
                        for i in range(tg + 1):
                            nc.tensor.matmul(
                                av[:, h, :],
                                p[:, i, h, tl * P : (tl + 1) * P],
                                v_ext[:, i, 2 * j + h, :],
                                start=(i == 0),
                                stop=(i == tg),
                            )
                    recip = smallp.tile([P, 2, 1], F32, tag="recip", name="recip")
                    nc.vector.reciprocal(recip[:], av[:, :, D : D + 1])
                    y_pair = smallp.tile([P, 2, D], BF16, tag="yp", name="y_pair")
                    nc.vector.tensor_mul(
                        out=y_pair[:],
                        in0=av[:, :, 0:D],
                        in1=recip[:].to_broadcast((P, 2, D)),
                    )
                    ytr = av_ps.tile([P, P], BF16, tag="av", name=f"ytr{j}{w}{tl}")
                    nc.tensor.transpose(
                        ytr[:], y_pair[:].rearrange("p h d -> p (h d)"), ident_bf[:]
                    )
                    nc.vector.tensor_copy(
                        out=yT[:, j, tg * P : (tg + 1) * P], in_=ytr[:]
                    )

            def proj(rc):
                # full-C z rows (2KB descriptors), one DMA per tt for
                # pipelining; late tts alternate DGE rings
                for idx, tt in enumerate((2 * rc, 2 * rc + 1)):
                    z_sb = zoutp.tile([P, C], BF16, tag="z", name=f"z{rc}{idx}")
                    for n in range(2):
                        ps = mm_ps.tile([P, 512], F32, tag="mm", name=f"pj{rc}{tt}{n}")
                        for c in range(HC // P):
                            nc.tensor.matmul(
                                ps[:],
                                yT[:, c, tt * P : (tt + 1) * P],
                                wp_sb[:, c, n * 512 : (n + 1) * 512],
                                start=(c == 0),
                                stop=(c == HC // P - 1),
                            )
                        nc.vector.tensor_add(
                            out=z_sb[:, n * 512 : (n + 1) * 512],
                            in0=ps[:],
                            in1=bp_sb[:, n * 512 : (n + 1) * 512],
                        )
                    zt, zoff = z_chunk(tt * P)
                    eng = nc.scalar if (tt % 2 == 1 and tt >= 8) else nc.sync
                    eng.dma_start(zt[zoff : zoff + P, :], z_sb[:])

            def rs(row0, nrows):
                # ReduceScatter rows [row0, row0+nrows): even core gets the
                # first half, odd core the second; lands at out_d[row0//2:]
                o0 = row0 // 2
                oh = nrows // 2
                nc.gpsimd.collective_compute(
                    "ReduceScatter",
                    mybir.AluOpType.add,
                    replica_groups=[[0, 1], [2, 3], [4, 5], [6, 7]],
                    ins=[z_drams[row0][:].opt()],
                    outs=[rs_out[o0 : o0 + oh, :].opt()],
                )
                nc.gpsimd.dma_start(
                    out_d[o0 : o0 + oh, :],
                    rs_out[o0 : o0 + oh, :],
                )

            # graduated RS chunks: big ones overlap compute, the last is small
            for w in range(NW):
                for j in range(NP):
                    unit(j, w)
                proj(2 * w)
                proj(2 * w + 1)
                if w == 1:
                    rs(0, 1024)
                elif w == 2:
                    rs(1024, 512)
                elif w == 3:
                    rs(1536, 256)
                    rs(1792, 256)

    nc.compile()
    return nc


def _in_maps(inputs):
    x = np.asarray(inputs["x"], dtype=np.float32)
    w_attn = np.asarray(inputs["w_attn"], dtype=np.float32)
    b_attn = np.asarray(inputs["b_attn"], dtype=np.float32)
    w_proj = np.asarray(inputs["w_proj"], dtype=np.float32)
    b_proj = np.asarray(inputs["b_proj"], dtype=np.float32)

    maps = []
    for core in range(N_CORES):
        b, g = core // 2, core % 2
        s = g * HC
        # x [T, C] -> x^T [ci, ck, t] with c = ck*128+ci
        xT = (
            x[b]
            .T.reshape(CK, P, NW, WIN)
            .transpose(2, 1, 0, 3)
            .astype(ml_dtypes.bfloat16)
        )
        # [C, HC] -> [ki, j, ko, n] with c = ko*128+ki, qcol = j*128+n
        wq = (
            w_attn[:, s : s + HC]
            .reshape(CK, P, NP, P)
            .transpose(1, 2, 0, 3)
            .astype(ml_dtypes.bfloat16)
        )
        wk = (
            w_attn[:, C + s : C + s + HC]
            .reshape(CK, P, NP, P)
            .transpose(1, 2, 0, 3)
            .astype(ml_dtypes.bfloat16)
        )
        # [C, HC] -> [ki, ko, vcol]
        wv = (
            w_attn[:, 2 * C + s : 2 * C + s + HC]
            .reshape(CK, P, HC)
            .transpose(1, 0, 2)
            .astype(ml_dtypes.bfloat16)
        )
        # [HC, C] -> [ki, ko, co]
        wp = (
            w_proj[s : s + HC, :]
            .reshape(HC // P, P, C)
            .transpose(1, 0, 2)
            .astype(ml_dtypes.bfloat16)
        )
        bq = b_attn[s : s + HC].reshape(NP, P).T
        bk = b_attn[C + s : C + s + HC].reshape(NP, P).T
        bv = np.broadcast_to(
            b_attn[2 * C + s : 2 * C + s + HC].astype(ml_dtypes.bfloat16), (P, HC)
        )
        bp = (
            np.broadcast_to(b_proj.astype(ml_dtypes.bfloat16), (P, C))
            if g == 0
            else np.zeros((P, C), ml_dtypes.bfloat16)
        )
        maps.append(
            {
                "xT": np.ascontiguousarray(xT),
                "wq": np.ascontiguousarray(wq),
                "wk": np.ascontiguousarray(wk),
                "wv": np.ascontiguousarray(wv),
                "wp": np.ascontiguousarray(wp),
                "bq": np.ascontiguousarray(bq),
                "bk": np.ascontiguousarray(bk),
                "bv": np.ascontiguousarray(bv),
                "bp": np.ascontiguousarray(bp),
            }
        )
    return maps


def _run(inputs, trace=False, trace_cores=None):
    if "nc" not in _CACHE:
        _CACHE["nc"] = _build_nc()
    nc = _CACHE["nc"]
    res = run_bass_kernel_spmd(
        nc,
        _in_maps(inputs),
        list(range(N_CORES)),
        trace=trace,
        trace_cores=trace_cores,
    )
    # RS ownership per chunk (row0, nrows): even core holds the first
    # nrows/2 rows, odd core the second half, at out row row0//2
    out = np.empty((B, T, C), np.float32)
    for b in range(B):
        ev = res.results[2 * b]["out"].astype(np.float32)
        od = res.results[2 * b + 1]["out"].astype(np.float32)
        for row0, nrows in ((0, 1024), (1024, 512), (1536, 256), (1792, 256)):
            o0, oh = row0 // 2, nrows // 2
            out[b, row0 : row0 + oh] = ev[o0 : o0 + oh]
            out[b, row0 + oh : row0 + nrows] = od[o0 : o0 + oh]
    return out, res


def kernel(**inputs):
    out, _ = _run(inputs)
    return out


# revision 23
# speedup vs baseline: 1.1939x; 1.1939x over previous
"""Causal self-attention (B=4, T=2048, C=1024, H=16) on 8 Trainium2 cores.

Sharding: core c -> batch b = c//2, head-group g = c%2 (8 heads each,
tensor-parallel). QKV + attention + c_proj computed per core on its head
slice; partial c_proj outputs of a (b) pair are summed with chunked
on-device ReduceScatters over the T dimension; host reassembles.

x arrives pre-transposed/pre-cast ([C, T] bf16) from the host, so the
kernel starts matmuls as soon as the first t-slice lands. Attention is
pipelined per (head-pair, 1024-query-block) unit: score matmuls (PE) ->
exp (Scalar, both heads per ACT) -> causal diag zeroing (GpSimd) ->
p@v chains (PE) -> normalize (Vector recip + GpSimd mul) -> y^T
(PE transpose + GpSimd copy). Projection + ReduceScatter fire in 8
chunks of 256 rows as soon as each row range is complete.

Self-contained: only imports concourse (installed library) + numpy.
"""

import ml_dtypes
import numpy as np

import concourse.mybir as mybir
import concourse.tile as tile
from concourse import bacc
from concourse.bass_utils import run_bass_kernel_spmd
from concourse.masks import make_identity

B, T, C = 4, 2048, 1024
H_TOTAL, D = 16, 64
N_CORES = 8
HL = H_TOTAL // 2  # local heads per core (8)
HC = HL * D  # local head cols (512)
NP = HL // 2  # head pairs (4)
P = 128
TT = T // P  # 16 t-chunks of 128
CK = C // P  # 8 contraction chunks for qkv
WIN = 512
NW = T // WIN  # 4 query windows of 512
NRC = 8  # ReduceScatter chunks (256 rows in, 128 out each)
F32 = mybir.dt.float32
BF16 = mybir.dt.bfloat16
SCALE = 1.0 / 8.0  # 1/sqrt(D)
MASK_VAL = -480.0  # -60 after the 1/8 attention scale; exp(-60) ~ 0

_CACHE = {}


def _build_nc():
    nc = bacc.Bacc("TRN2", target_bir_lowering=False, debug=False, num_devices=N_CORES)

    xT_d = nc.dram_tensor("xT", [NW, P, CK, WIN], BF16, kind="ExternalInput")
    wq_d = nc.dram_tensor("wq", [P, NP, CK, P], BF16, kind="ExternalInput")
    wk_d = nc.dram_tensor("wk", [P, NP, CK, P], BF16, kind="ExternalInput")
    wv_d = nc.dram_tensor("wv", [P, CK, HC], BF16, kind="ExternalInput")
    bq_d = nc.dram_tensor("bq", [P, NP], F32, kind="ExternalInput")
    bk_d = nc.dram_tensor("bk", [P, NP], F32, kind="ExternalInput")
    bv_d = nc.dram_tensor("bv", [P, HC], BF16, kind="ExternalInput")
    wp_d = nc.dram_tensor("wp", [P, HC // P, C], BF16, kind="ExternalInput")
    bp_d = nc.dram_tensor("bp", [P, C], BF16, kind="ExternalInput")
    out_d = nc.dram_tensor("out", [T // 2, C], BF16, kind="ExternalOutput")

    with tile.TileContext(nc) as tc:
        with (
            tc.tile_pool(name="const", bufs=1) as constp,
            tc.tile_pool(name="big", bufs=1) as bigp,
            tc.tile_pool(name="pp", bufs=2) as pp,
            tc.tile_pool(name="small", bufs=2) as smallp,
            tc.tile_pool(name="zout", bufs=2) as zoutp,
            tc.tile_pool(name="score_ps", bufs=2, space="PSUM") as score_ps,
            tc.tile_pool(name="av_ps", bufs=2, space="PSUM") as av_ps,
            tc.tile_pool(name="mm_ps", bufs=2, space="PSUM") as mm_ps,
            tc.tile_pool(name="dram", bufs=1, space="DRAM") as dramp,
        ):
            # ---- constants ----
            ident = constp.tile([P, P], F32)
            make_identity(nc, ident)
            ident_bf = constp.tile([P, P], BF16)
            nc.vector.tensor_copy(out=ident_bf[:], in_=ident[:])
            # multiplicative causal mask for the diagonal 128x128 block:
            # trimask[k, q] = 1 where q >= k else 0
            trif = constp.tile([P, P], F32)
            nc.gpsimd.memset(trif, 1.0)
            nc.gpsimd.affine_select(
                out=trif,
                in_=trif,
                compare_op=mybir.AluOpType.is_ge,
                fill=0.0,
                base=0,
                pattern=[[1, P]],
                channel_multiplier=-1,
            )
            trimask = constp.tile([P, P], BF16)
            nc.vector.tensor_copy(out=trimask[:], in_=trif[:])
            bq_sb = constp.tile([P, NP], F32)
            bk_sb = constp.tile([P, NP], F32)
            bv_sb = constp.tile([P, HC], BF16)
            bp_sb = constp.tile([P, C], BF16)

            # ---- persistent activations / weights ----
            xT = bigp.tile([P, NW, CK, WIN], BF16)  # x^T [c, (slice, ck, t)]
            qT = bigp.tile([P, NP, T], BF16)  # q^T [qcol, t]
            kT = bigp.tile([P, NP, T], BF16)  # k^T [kcol, t]
            v_ext = bigp.tile([P, TT, HL, D + 1], BF16)  # v with ones col
            yT = bigp.tile([P, NP, T], BF16)  # y^T [ci, t]
            wv_sb = bigp.tile([P, CK, HC], BF16)
            wp_sb = bigp.tile([P, HC // P, C], BF16)
            wq_sb = bigp.tile([P, NP, CK, P], BF16)
            wk_sb = bigp.tile([P, NP, CK, P], BF16)

            # spread the startup DMAs over independent engine DGE rings so
            # the first v/qk chains are gated only by xT slice 0 + its weights:
            # scalar ring: wv first; gpsimd ring: wq first; sync ring: xT slices
            nc.scalar.dma_start(wv_sb[:], wv_d[:])
            nc.gpsimd.dma_start(wq_sb[:], wq_d[:])
            nc.scalar.dma_start(wk_sb[:], wk_d[:])
            nc.gpsimd.dma_start(wp_sb[:], wp_d[:])
            nc.scalar.dma_start(bv_sb[:], bv_d[:])
            nc.scalar.dma_start(bq_sb[:], bq_d[:])
            nc.scalar.dma_start(bk_sb[:], bk_d[:])
            nc.gpsimd.dma_start(bp_sb[:], bp_d[:])
            nc.vector.memset(v_ext[:, :, :, D : D + 1], 1.0)

            # one DRAM staging tensor per RS chunk so each collective
            # depends only on its own rows' writes
            rs_rows = ((0, 1024), (1024, 512), (1536, 512))
            z_drams = {
                r0: dramp.tile([nr, C], BF16, name=f"z_dram{r0}")
                for r0, nr in rs_rows
            }
            rs_out = dramp.tile([T // 2, C], BF16)

            def z_chunk(row0):
                for r0, nr in rs_rows:
                    if r0 <= row0 < r0 + nr:
                        return z_drams[r0], row0 - r0
                raise AssertionError

            # ---- phase 1: qkv projections, pipelined on xT t-slices ----
            for s in range(NW):
                nc.sync.dma_start(xT[:, s], xT_d[s])
                for tt in range(4 * s, 4 * s + 4):
                    ps = mm_ps.tile([P, 512], F32, tag="mm", name=f"v_ps{tt}")
                    for ck in range(CK):
                        nc.tensor.matmul(
                            ps[:],
                            xT[:, tt // 4, ck, (tt % 4) * P : (tt % 4 + 1) * P],
                            wv_sb[:, ck, :],
                            start=(ck == 0),
                            stop=(ck == CK - 1),
                        )
                    nc.vector.tensor_add(
                        out=v_ext[:, tt, :, 0:D],
                        in0=ps[:].rearrange("p (h d) -> p h d", d=D),
                        in1=bv_sb[:].rearrange("p (h d) -> p h d", d=D),
                    )
                for j in range(NP):
                    for w_sb, b_sb, dstT in (
                        (wq_sb, bq_sb, qT),
                        (wk_sb, bk_sb, kT),
                    ):
                        ps = mm_ps.tile([P, 512], F32, tag="mm", name=f"qk_ps{s}{j}")
                        for ck in range(CK):
                            nc.tensor.matmul(
                                ps[:],
                                w_sb[:, j, ck, :],
                                xT[:, s, ck, :],
                                start=(ck == 0),
                                stop=(ck == CK - 1),
                            )
                        nc.vector.tensor_add(
                            out=dstT[:, j, s * WIN : (s + 1) * WIN],
                            in0=ps[:],
                            in1=b_sb[:, j : j + 1].to_broadcast((P, WIN)),
                        )

            # ---- attention units + chunked proj/RS ----
            def unit(j, u):
                qbase = u * 1024
                pw = [
                    pp.tile([P, TT, 2, WIN], BF16, tag="p", name=f"p{j}{u}{w}")
                    for w in range(2)
                ]
                for i in range(8 * (u + 1)):
                    for w in range(2):
                        c0r = i * P - (qbase + w * WIN)
                        if c0r >= WIN:
                            continue
                        c0 = max(0, c0r)
                        sp = score_ps.tile(
                            [P, 2, WIN], F32, tag="sc", name=f"sp{j}{u}{i}{w}"
                        )
                        for h in range(2):
                            hb = h * D
                            nc.tensor.matmul(
                                sp[:, h, c0:WIN],
                                kT[hb : hb + D, j, i * P : (i + 1) * P],
                                qT[hb : hb + D, j, qbase + w * WIN + c0 : qbase + (w + 1) * WIN],
                                start=True,
                                stop=True,
                            )
                        nc.scalar.activation(
                            out=pw[w][:, i, :, c0:WIN],
                            in_=sp[:, :, c0:WIN],
                            func=mybir.ActivationFunctionType.Exp,
                            scale=SCALE,
                        )
                        if c0r >= 0:
                            # zero the strictly-upper triangle of the
                            # diagonal block (future keys) post-exp
                            nc.vector.tensor_mul(
                                out=pw[w][:, i, :, c0 : c0 + P],
                                in0=pw[w][:, i, :, c0 : c0 + P],
                                in1=trimask[:, None, :].to_broadcast((P, 2, P)),
                            )
                for tl in range(8):
                    tg = u * 8 + tl
                    p = pw[tl // 4]
                    tc = tl % 4
                    av = av_ps.tile([P, 2, D + 1], F32, tag="av", name=f"av{j}{u}{tl}")
                    for h in range(2):
                        for i in range(tg + 1):
                            nc.tensor.matmul(
                                av[:, h, :],
                                p[:, i, h, tc * P : (tc + 1) * P],
                                v_ext[:, i, 2 * j + h, :],
                                start=(i == 0),
                                stop=(i == tg),
                            )
                    recip = smallp.tile([P, 2, 1], F32, tag="recip", name="recip")
                    nc.vector.reciprocal(recip[:], av[:, :, D : D + 1])
                    y_pair = smallp.tile([P, 2, D], BF16, tag="yp", name="y_pair")
                    nc.vector.tensor_mul(
                        out=y_pair[:],
                        in0=av[:, :, 0:D],
                        in1=recip[:].to_broadcast((P, 2, D)),
                    )
                    ytr = av_ps.tile([P, P], BF16, tag="av", name=f"ytr{j}{u}{tl}")
                    nc.tensor.transpose(
                        ytr[:], y_pair[:].rearrange("p h d -> p (h d)"), ident_bf[:]
                    )
                    nc.vector.tensor_copy(
                        out=yT[:, j, tg * P : (tg + 1) * P], in_=ytr[:]
                    )

            def proj(rc):
                # full-C z rows (2KB descriptors), one DMA per tt for
                # pipelining; late tts alternate DGE rings
                for idx, tt in enumerate((2 * rc, 2 * rc + 1)):
                    z_sb = zoutp.tile([P, C], BF16, tag="z", name=f"z{rc}{idx}")
                    for n in range(2):
                        ps = mm_ps.tile([P, 512], F32, tag="mm", name=f"pj{rc}{tt}{n}")
                        for c in range(HC // P):
                            nc.tensor.matmul(
                                ps[:],
                                yT[:, c, tt * P : (tt + 1) * P],
                                wp_sb[:, c, n * 512 : (n + 1) * 512],
                                start=(c == 0),
                                stop=(c == HC // P - 1),
                            )
                        nc.vector.tensor_add(
                            out=z_sb[:, n * 512 : (n + 1) * 512],
                            in0=ps[:],
                            in1=bp_sb[:, n * 512 : (n + 1) * 512],
                        )
                    zt, zoff = z_chunk(tt * P)
                    eng = nc.scalar if (tt % 2 == 1 and tt >= 8) else nc.sync
                    eng.dma_start(zt[zoff : zoff + P, :], z_sb[:])

            def rs(row0, nrows):
                # ReduceScatter rows [row0, row0+nrows): even core gets the
                # first half, odd core the second; lands at out_d[row0//2:]
                o0 = row0 // 2
                oh = nrows // 2
                nc.gpsimd.collective_compute(
                    "ReduceScatter",
                    mybir.AluOpType.add,
                    replica_groups=[[0, 1], [2, 3], [4, 5], [6, 7]],
                    ins=[z_drams[row0][:].opt()],
                    outs=[rs_out[o0 : o0 + oh, :].opt()],
                )
                nc.gpsimd.dma_start(
                    out_d[o0 : o0 + oh, :],
                    rs_out[o0 : o0 + oh, :],
                )

            # graduated RS chunks: u=0 RS fully overlapped; u=1 in two
            # 512-row pieces so the first overlaps the end of compute
            for u in range(2):
                for j in range(NP):
                    unit(j, u)
                for rc in range(4 * u, 4 * u + 4):
                    proj(rc)
                    if u == 1 and rc == 5:
                        rs(1024, 512)
                if u == 0:
                    rs(0, 1024)
                else:
                    rs(1536, 512)

    nc.compile()
    return nc


def _in_maps(inputs):
    x = np.asarray(inputs["x"], dtype=np.float32)
    w_attn = np.asarray(inputs["w_attn"], dtype=np.float32)
    b_attn = np.asarray(inputs["b_attn"], dtype=np.float32)
    w_proj = np.asarray(inputs["w_proj"], dtype=np.float32)
    b_proj = np.asarray(inputs["b_proj"], dtype=np.float32)

    maps = []
    for core in range(N_CORES):
        b, g = core // 2, core % 2
        s = g * HC
        # x [T, C] -> x^T [ci, ck, t] with c = ck*128+ci
        xT = (
            x[b]
            .T.reshape(CK, P, NW, WIN)
            .transpose(2, 1, 0, 3)
            .astype(ml_dtypes.bfloat16)
        )
        # [C, HC] -> [ki, j, ko, n] with c = ko*128+ki, qcol = j*128+n
        wq = (
            w_attn[:, s : s + HC]
            .reshape(CK, P, NP, P)
            .transpose(1, 2, 0, 3)
            .astype(ml_dtypes.bfloat16)
        )
        wk = (
            w_attn[:, C + s : C + s + HC]
            .reshape(CK, P, NP, P)
            .transpose(1, 2, 0, 3)
            .astype(ml_dtypes.bfloat16)
        )
        # [C, HC] -> [ki, ko, vcol]
        wv = (
            w_attn[:, 2 * C + s : 2 * C + s + HC]
            .reshape(CK, P, HC)
            .transpose(1, 0, 2)
            .astype(ml_dtypes.bfloat16)
        )
        # [HC, C] -> [ki, ko, co]
        wp = (
            w_proj[s : s + HC, :]
            .reshape(HC // P, P, C)
            .transpose(1, 0, 2)
            .astype(ml_dtypes.bfloat16)
        )
        bq = b_attn[s : s + HC].reshape(NP, P).T
        bk = b_attn[C + s : C + s + HC].reshape(NP, P).T
        bv = np.broadcast_to(
            b_attn[2 * C + s : 2 * C + s + HC].astype(ml_dtypes.bfloat16), (P, HC)
        )
        bp = (
            np.broadcast_to(b_proj.astype(ml_dtypes.bfloat16), (P, C))
            if g == 0
            else np.zeros((P, C), ml_dtypes.bfloat16)
        )
        maps.append(
            {
                "xT": np.ascontiguousarray(xT),
                "wq": np.ascontiguousarray(wq),
                "wk": np.ascontiguousarray(wk),
                "wv": np.ascontiguousarray(wv),
                "wp": np.ascontiguousarray(wp),
                "bq": np.ascontiguousarray(bq),
                "bk": np.ascontiguousarray(bk),
                "bv": np.ascontiguousarray(bv),
                "bp": np.ascontiguousarray(bp),
            }
        )
    return maps


def _run(inputs, trace=False, trace_cores=None):
    if "nc" not in _CACHE:
        _CACHE["nc"] = _build_nc()
    nc = _CACHE["nc"]
    res = run_bass_kernel_spmd(
        nc,
        _in_maps(inputs),
        list(range(N_CORES)),
        trace=trace,
        trace_cores=trace_cores,
    )
    # RS ownership per chunk (row0, nrows): even core holds the first
    # nrows/2 rows, odd core the second half, at out row row0//2
    out = np.empty((B, T, C), np.float32)
    for b in range(B):
        ev = res.results[2 * b]["out"].astype(np.float32)
        od = res.results[2 * b + 1]["out"].astype(np.float32)
        for row0, nrows in ((0, 1024), (1024, 512), (1536, 512)):
            o0, oh = row0 // 2, nrows // 2
            out[b, row0 : row0 + oh] = ev[o0 : o0 + oh]
            out[b, row0 + oh : row0 + nrows] = od[o0 : o0 + oh]
    return out, res


def kernel(**inputs):
    out, _ = _run(inputs)
    return out
